# revision 20
# baseline (speedup 1.0000x reference)
"""Trainium2 Bass kernel for EntityAwareLSTMLayer.

Problem (hardcoded):
  B=1024, T=365, DYN=32, STATIC=27, UNITS=256
  i_gate = sigmoid(x_static @ W_sh + bias_s)            [B, U]   (static, once)
  gx_t   = x_t @ W_ih + bias                            [B, 3U]
  gates  = gx_t + h @ W_hh                              [B, 3U]  (f|o|g)
  c      = sigmoid(f) * c + i_gate * tanh(g)
  h      = sigmoid(o) * tanh(c)
  return h_final                                        [B, U]

Sharding: data-parallel over batch, 8 cores x 128 rows. Batch rows live on
the 128 SBUF partitions; per step the gates are computed by PE matmuls
accumulating three K-chunks into PSUM: the x chunk (K=32) and two h chunks
(K=128 each, h stored transposed). fp16 matmul operands (1 cycle/row on PE
vs 4 for fp32), fp32 PSUM accumulation, fp32 cell state.

x_dynamic is transposed on-chip via DMA-xbar transposes of [128,128] fp16
chunks (4 timesteps per chunk); timestep t lands at partition group
32*(t%4), so W_ih is replicated at the 4 partition bases.
"""

import numpy as np

B_L = 128  # batch rows per core
T = 365
TP = 368  # T padded to a multiple of 4 for chunked transposes
DYN = 32
STATIC = 27
U = 256
NCORES = 8

_cached = {}


def _build_program(has_bias: bool):
    from contextlib import ExitStack

    import concourse.bacc as bacc
    import concourse.masks as masks
    import concourse.tile as tile
    from concourse import mybir

    f32 = mybir.dt.float32
    f16 = mybir.dt.float16
    AF = mybir.ActivationFunctionType

    nc = bacc.Bacc("TRN2", target_bir_lowering=False, debug=False)

    x_dyn = nc.dram_tensor("x_dynamic", [B_L, T * DYN], f32, kind="ExternalInput")
    x_st = nc.dram_tensor("x_static", [B_L, STATIC], f32, kind="ExternalInput")
    w_ih = nc.dram_tensor("weight_ih", [DYN, 3 * U], f32, kind="ExternalInput")
    w_hh = nc.dram_tensor("weight_hh", [U, 3 * U], f32, kind="ExternalInput")
    w_sh = nc.dram_tensor("weight_sh", [STATIC, U], f32, kind="ExternalInput")
    bias = nc.dram_tensor("bias", [1, 3 * U], f32, kind="ExternalInput")
    bias_s = nc.dram_tensor("bias_s", [1, U], f32, kind="ExternalInput")
    out = nc.dram_tensor("out", [B_L, U], f32, kind="ExternalOutput")

    with tile.TileContext(nc) as tc, ExitStack() as ctx:
        const = ctx.enter_context(tc.tile_pool(name="const", bufs=1))
        # x transposed in 4-timestep chunks: chunk c = timesteps 4c..4c+3,
        # partition 32*(t%4)+k holds feature k of timestep t, free col = batch.
        xT4 = const.tile([128, (TP // 4) * B_L], f16)
        Wih4 = const.tile([128, 3 * U], f16)  # W_ih replicated at 4 bases
        Whh0 = const.tile([128, 3 * U], f16)
        Whh1 = const.tile([128, 3 * U], f16)
        Wshb = const.tile([STATIC + 1, U], f16)  # rows 0-26 W_sh, row 27 bias_s
        xsT = const.tile([128, B_L], f16)
        ident = const.tile([128, 128], f16)
        igate = const.tile([128, U], f16)
        if has_bias:
            ones_row = const.tile([1, B_L], f16)
            bias16 = const.tile([1, 3 * U], f16)

        psum_fo = ctx.enter_context(tc.tile_pool(name="pfo", bufs=3, space="PSUM"))
        psum_g = ctx.enter_context(tc.tile_pool(name="pg", bufs=3, space="PSUM"))
        psum_t = ctx.enter_context(tc.tile_pool(name="pt", bufs=2, space="PSUM"))

        with tc.tile_pool(name="stage", bufs=1) as stage:
            # --- x_dynamic: load fp32, convert fp16, transpose in chunks ---
            xs32 = stage.tile([B_L, T * DYN], f32)
            nc.sync.dma_start(xs32[:], x_dyn[:])
            x16 = stage.tile([B_L, TP * DYN], f16)
            nc.vector.memset(x16[:, T * DYN :], 0.0)
            nc.vector.tensor_copy(x16[:, 0 : T * DYN], xs32[:])
            for c in range(TP // 4):
                nc.sync.dma_start_transpose(
                    xT4[:, c * B_L : (c + 1) * B_L],
                    x16[:, c * 128 : (c + 1) * 128],
                )

            # --- weights ---
            wst = stage.tile([128, 3 * U], f32)
            nc.sync.dma_start(wst[:], w_hh[0:128, :])
            nc.vector.tensor_copy(Whh0[:], wst[:])
            nc.sync.dma_start(wst[:], w_hh[128:256, :])
            nc.vector.tensor_copy(Whh1[:], wst[:])
            wih32 = stage.tile([DYN, 3 * U], f32)
            nc.sync.dma_start(wih32[:], w_ih[:])
            for g in range(4):
                nc.vector.tensor_copy(Wih4[32 * g : 32 * g + 32, :], wih32[:])
            wsh32 = stage.tile([STATIC, U], f32)
            nc.sync.dma_start(wsh32[:], w_sh[:])
            nc.vector.tensor_copy(Wshb[0:STATIC, :], wsh32[:])
            bs32 = stage.tile([1, U], f32)
            nc.sync.dma_start(bs32[:], bias_s[:])
            bs16 = stage.tile([1, U], f16)
            nc.vector.tensor_copy(bs16[:], bs32[:])
            # partition 27 is not engine-addressable; DMA has no such limit
            nc.sync.dma_start(Wshb[STATIC : STATIC + 1, :], bs16[:])
            if has_bias:
                b32 = stage.tile([1, 3 * U], f32)
                nc.sync.dma_start(b32[:], bias[:])
                nc.vector.tensor_copy(bias16[:], b32[:])
                nc.vector.memset(ones_row[:], 1.0)

            # --- x_static -> transposed [27, 128] + ones row 27 ---
            xst32 = stage.tile([B_L, STATIC], f32)
            nc.sync.dma_start(xst32[:], x_st[:])
            xst16 = stage.tile([B_L, 128], f16)
            nc.vector.memset(xst16[:], 0.0)
            nc.vector.tensor_copy(xst16[:, 0:STATIC], xst32[:])
            # ones in column 27 become the ones row after the transpose
            nc.vector.memset(xst16[:, STATIC : STATIC + 1], 1.0)
            nc.sync.dma_start_transpose(xsT[:], xst16[:])

            masks.make_identity(nc, ident[:])

            # --- i_gate = sigmoid(x_static @ W_sh + bias_s) ---
            ig_ps = psum_g.tile([128, U], f32, tag="ps_g")
            nc.tensor.matmul(
                ig_ps[:], xsT[0 : STATIC + 1, :], Wshb[:], start=True, stop=True
            )
            nc.scalar.activation(igate[:], ig_ps[:], AF.Sigmoid)

        # --- recurrent state ---
        st = ctx.enter_context(tc.tile_pool(name="state", bufs=2))
        tmp = ctx.enter_context(tc.tile_pool(name="tmp", bufs=3))

        c_prev = st.tile([128, U], f32, tag="c")
        nc.vector.memset(c_prev[:], 0.0)
        hT0 = st.tile([128, B_L], f16, tag="h0")
        nc.vector.memset(hT0[:], 0.0)
        hT1 = st.tile([128, B_L], f16, tag="h1")
        nc.vector.memset(hT1[:], 0.0)

        def x_matmuls(t, ps_fo, ps_g):
            g4 = 32 * (t % 4)
            cc = t // 4
            xt = xT4[g4 : g4 + 32, cc * B_L : (cc + 1) * B_L]
            wx = Wih4[g4 : g4 + 32, :]
            nc.tensor.matmul(
                ps_fo[:],
                xt,
                wx[:, 0 : 2 * U],
                start=True,
                stop=False,
                tile_position=(g4, 0),
            )
            nc.tensor.matmul(
                ps_g[:],
                xt,
                wx[:, 2 * U : 3 * U],
                start=True,
                stop=False,
                tile_position=(g4, 0),
            )
            if has_bias:
                nc.tensor.matmul(
                    ps_fo[:], ones_row[:], bias16[:, 0 : 2 * U], start=False, stop=False
                )
                nc.tensor.matmul(
                    ps_g[:],
                    ones_row[:],
                    bias16[:, 2 * U : 3 * U],
                    start=False,
                    stop=False,
                )

        # Warmup burst: ~16 back-to-back matmuls give the PE HAM monitor a
        # sustained-busy window so it lifts the clock gate to 2.4 GHz before
        # the scan starts; steady-state gaps are too short to re-throttle.
        for w in range(16):
            wp = psum_t.tile([128, 2 * U], f32, tag="pt")
            nc.tensor.matmul(wp[:], ident[:], Whh0[:, 0 : 2 * U], start=True, stop=True)

        # software-pipelined by two steps: the x contributions for steps t+1
        # and t+2 are issued during step t (they depend only on constants),
        # which fills PE idle gaps (keeping the HAM clock warm) and leaves
        # only the h matmuls on the step-boundary critical path.
        pipe = []
        for tt in range(min(2, T)):
            ps_fo_n = psum_fo.tile([128, 2 * U], f32, tag="ps_fo")
            ps_g_n = psum_g.tile([128, U], f32, tag="ps_g")
            x_matmuls(tt, ps_fo_n, ps_g_n)
            pipe.append((ps_fo_n, ps_g_n))

        for t in range(T):
            last = t == T - 1
            ps_fo, ps_g = pipe.pop(0)
            nc.tensor.matmul(
                ps_fo[:], hT0[:], Whh0[:, 0 : 2 * U], start=False, stop=False
            )
            nc.tensor.matmul(
                ps_fo[:], hT1[:], Whh1[:, 0 : 2 * U], start=False, stop=True
            )
            nc.tensor.matmul(
                ps_g[:], hT0[:], Whh0[:, 2 * U : 3 * U], start=False, stop=False
            )
            nc.tensor.matmul(
                ps_g[:], hT1[:], Whh1[:, 2 * U : 3 * U], start=False, stop=True
            )
            if t + 2 < T:
                ps_fo_n = psum_fo.tile([128, 2 * U], f32, tag="ps_fo")
                ps_g_n = psum_g.tile([128, U], f32, tag="ps_g")
                x_matmuls(t + 2, ps_fo_n, ps_g_n)
                pipe.append((ps_fo_n, ps_g_n))
            if not last:
                # HAM filler: keep the PE activity monitor busy through the
                # elementwise window so the 2.4 GHz clock state sticks
                wp = psum_t.tile([128, 2 * U], f32, tag="pt")
                nc.tensor.matmul(
                    wp[:], ident[:], Whh0[:, 0 : 2 * U], start=True, stop=True
                )

            # sigmoid(f) alone first: it gates the c chain; sigmoid(o) is only
            # needed ~1.5us later for the h products.
            sf = tmp.tile([128, U], f16, tag="sf")
            nc.scalar.activation(sf[:], ps_fo[:, 0:U], AF.Sigmoid)
            tg = tmp.tile([128, U], f16, tag="tg")
            nc.scalar.activation(tg[:], ps_g[:], AF.Tanh)
            so = tmp.tile([128, U], f16, tag="so")
            nc.scalar.activation(so[:], ps_fo[:, U : 2 * U], AF.Sigmoid)

            m1 = tmp.tile([128, U], f32, tag="m1")
            nc.vector.tensor_mul(m1[:], sf[:], c_prev[:])
            m2 = tmp.tile([128, U], f16, tag="m2")
            nc.vector.tensor_mul(m2[:], igate[:], tg[:])
            c_new = st.tile([128, U], f32, tag="c")
            nc.vector.tensor_add(c_new[:], m1[:], m2[:])

            if last:
                tch = tmp.tile([128, U], f32, tag="tc32")
                nc.scalar.activation(tch[:], c_new[:], AF.Tanh)
                h_out = tmp.tile([128, U], f32, tag="hout")
                nc.vector.tensor_mul(h_out[:], so[:], tch[:])
                nc.sync.dma_start(out[:], h_out[:])
            else:
                # tail split into u-halves so transpose/copy/h-matmul of half 0
                # start while half 1 is still in ACT/DVE
                hTn = [None, None]
                for half in (0, 1):
                    lo, hi = 128 * half, 128 * (half + 1)
                    tch = tmp.tile([128, 128], f16, tag=f"tc{half}")
                    nc.scalar.activation(tch[:], c_new[:, lo:hi], AF.Tanh)
                    hh = tmp.tile([128, 128], f16, tag=f"hh{half}")
                    nc.vector.tensor_mul(hh[:], so[:, lo:hi], tch[:])
                    pp = psum_t.tile([128, 128], f16, tag="pt")
                    nc.tensor.transpose(pp[:], hh[:], ident[:])
                    ht_new = st.tile([128, B_L], f16, tag=f"h{half}")
                    nc.scalar.copy(ht_new[:], pp[:])
                    hTn[half] = ht_new
                hT0, hT1 = hTn
                wp2 = psum_t.tile([128, 2 * U], f32, tag="pt")
                nc.tensor.matmul(
                    wp2[:], ident[:], Whh0[:, 0 : 2 * U], start=True, stop=True
                )
                wp3 = psum_t.tile([128, 2 * U], f32, tag="pt")
                nc.tensor.matmul(
                    wp3[:], ident[:], Whh1[:, 0 : 2 * U], start=True, stop=True
                )
            c_prev = c_new

    nc.compile()
    return nc


def get_program(has_bias: bool = False):
    if has_bias not in _cached:
        _cached[has_bias] = _build_program(has_bias)
    return _cached[has_bias]


def make_in_maps(inputs):
    x_dynamic = np.asarray(inputs["x_dynamic"], dtype=np.float32)
    x_static = np.asarray(inputs["x_static"], dtype=np.float32)
    w_ih = np.ascontiguousarray(np.asarray(inputs["weight_ih"], dtype=np.float32))
    w_hh = np.ascontiguousarray(np.asarray(inputs["weight_hh"], dtype=np.float32))
    w_sh = np.ascontiguousarray(np.asarray(inputs["weight_sh"], dtype=np.float32))
    bias = np.ascontiguousarray(
        np.asarray(inputs["bias"], dtype=np.float32).reshape(1, 3 * U)
    )
    bias_s = np.ascontiguousarray(
        np.asarray(inputs["bias_s"], dtype=np.float32).reshape(1, U)
    )
    in_maps = []
    for i in range(NCORES):
        sl = slice(i * B_L, (i + 1) * B_L)
        in_maps.append(
            {
                "x_dynamic": np.ascontiguousarray(
                    x_dynamic[sl].reshape(B_L, T * DYN)
                ),
                "x_static": np.ascontiguousarray(x_static[sl]),
                "weight_ih": w_ih,
                "weight_hh": w_hh,
                "weight_sh": w_sh,
                "bias": bias,
                "bias_s": bias_s,
            }
        )
    return in_maps


def kernel(**inputs) -> np.ndarray:
    from concourse.bass_utils import run_bass_kernel_spmd

    has_bias = bool(np.any(np.asarray(inputs["bias"])))
    nc = get_program(has_bias)
    in_maps = make_in_maps(inputs)
    res = run_bass_kernel_spmd(nc, in_maps, core_ids=list(range(NCORES)))
    return np.concatenate([r["out"] for r in res.results], axis=0).astype(np.float32)


# revision 24
# speedup vs baseline: 1.1653x; 1.1653x over previous
"""Trainium2 Bass kernel for EntityAwareLSTMLayer.

Problem (hardcoded):
  B=1024, T=365, DYN=32, STATIC=27, UNITS=256
  i_gate = sigmoid(x_static @ W_sh + bias_s)            [B, U]   (static, once)
  gx_t   = x_t @ W_ih + bias                            [B, 3U]
  gates  = gx_t + h @ W_hh                              [B, 3U]  (f|o|g)
  c      = sigmoid(f) * c + i_gate * tanh(g)
  h      = sigmoid(o) * tanh(c)
  return h_final                                        [B, U]

Sharding: data-parallel over batch, 8 cores x 128 rows. Batch rows live on
the 128 SBUF partitions; per step the gates are computed by PE matmuls
accumulating three K-chunks into PSUM: the x chunk (K=32) and two h chunks
(K=128 each, h stored transposed). fp16 matmul operands (1 cycle/row on PE
vs 4 for fp32), fp32 PSUM accumulation, fp32 cell state.

x_dynamic is transposed on-chip via DMA-xbar transposes of [128,128] fp16
chunks (4 timesteps per chunk); timestep t lands at partition group
32*(t%4), so W_ih is replicated at the 4 partition bases.
"""

import numpy as np

B_L = 128  # batch rows per core
T = 365
TP = 368  # T padded to a multiple of 4 for chunked transposes
DYN = 32
STATIC = 27
U = 256
NCORES = 8

_cached = {}


def _build_program(has_bias: bool):
    from contextlib import ExitStack

    import concourse.bacc as bacc
    import concourse.masks as masks
    import concourse.tile as tile
    from concourse import mybir

    f32 = mybir.dt.float32
    f16 = mybir.dt.float16
    AF = mybir.ActivationFunctionType

    nc = bacc.Bacc("TRN2", target_bir_lowering=False, debug=False)

    x_dyn = nc.dram_tensor("x_dynamic", [B_L, T * DYN], f32, kind="ExternalInput")
    x_st = nc.dram_tensor("x_static", [B_L, STATIC], f32, kind="ExternalInput")
    w_ih = nc.dram_tensor("weight_ih", [DYN, 3 * U], f32, kind="ExternalInput")
    w_hh = nc.dram_tensor("weight_hh", [U, 3 * U], f32, kind="ExternalInput")
    w_sh = nc.dram_tensor("weight_sh", [STATIC, U], f32, kind="ExternalInput")
    bias = nc.dram_tensor("bias", [1, 3 * U], f32, kind="ExternalInput")
    bias_s = nc.dram_tensor("bias_s", [1, U], f32, kind="ExternalInput")
    out = nc.dram_tensor("out", [B_L, U], f32, kind="ExternalOutput")

    with tile.TileContext(nc) as tc, ExitStack() as ctx:
        const = ctx.enter_context(tc.tile_pool(name="const", bufs=1))
        # x transposed in 4-timestep chunks: chunk c = timesteps 4c..4c+3,
        # partition 32*(t%4)+k holds feature k of timestep t, free col = batch.
        xT4 = const.tile([128, (TP // 4) * B_L], f16)
        Wih4 = const.tile([128, 3 * U], f16)  # W_ih replicated at 4 bases
        Whh0 = const.tile([128, 3 * U], f16)
        Whh1 = const.tile([128, 3 * U], f16)
        Wshb = const.tile([STATIC + 1, U], f16)  # rows 0-26 W_sh, row 27 bias_s
        xsT = const.tile([128, B_L], f16)
        ident = const.tile([128, 128], f16)
        igate = const.tile([128, U], f16)
        if has_bias:
            ones_row = const.tile([1, B_L], f16)
            bias16 = const.tile([1, 3 * U], f16)

        psum_f = ctx.enter_context(tc.tile_pool(name="pf", bufs=2, space="PSUM"))
        psum_o = ctx.enter_context(tc.tile_pool(name="po", bufs=2, space="PSUM"))
        psum_g = ctx.enter_context(tc.tile_pool(name="pg", bufs=2, space="PSUM"))
        psum_t = ctx.enter_context(tc.tile_pool(name="pt", bufs=2, space="PSUM"))

        with tc.tile_pool(name="stage", bufs=1) as stage:
            # --- x_dynamic: load fp32, convert fp16, transpose in chunks ---
            xs32 = stage.tile([B_L, T * DYN], f32)
            nc.sync.dma_start(xs32[:], x_dyn[:])
            x16 = stage.tile([B_L, TP * DYN], f16)
            nc.vector.memset(x16[:, T * DYN :], 0.0)
            nc.vector.tensor_copy(x16[:, 0 : T * DYN], xs32[:])
            for c in range(TP // 4):
                nc.sync.dma_start_transpose(
                    xT4[:, c * B_L : (c + 1) * B_L],
                    x16[:, c * 128 : (c + 1) * 128],
                )

            # --- weights ---
            wst = stage.tile([128, 3 * U], f32)
            nc.sync.dma_start(wst[:], w_hh[0:128, :])
            nc.vector.tensor_copy(Whh0[:], wst[:])
            nc.sync.dma_start(wst[:], w_hh[128:256, :])
            nc.vector.tensor_copy(Whh1[:], wst[:])
            wih32 = stage.tile([DYN, 3 * U], f32)
            nc.sync.dma_start(wih32[:], w_ih[:])
            for g in range(4):
                nc.vector.tensor_copy(Wih4[32 * g : 32 * g + 32, :], wih32[:])
            wsh32 = stage.tile([STATIC, U], f32)
            nc.sync.dma_start(wsh32[:], w_sh[:])
            nc.vector.tensor_copy(Wshb[0:STATIC, :], wsh32[:])
            bs32 = stage.tile([1, U], f32)
            nc.sync.dma_start(bs32[:], bias_s[:])
            bs16 = stage.tile([1, U], f16)
            nc.vector.tensor_copy(bs16[:], bs32[:])
            # partition 27 is not engine-addressable; DMA has no such limit
            nc.sync.dma_start(Wshb[STATIC : STATIC + 1, :], bs16[:])
            if has_bias:
                b32 = stage.tile([1, 3 * U], f32)
                nc.sync.dma_start(b32[:], bias[:])
                nc.vector.tensor_copy(bias16[:], b32[:])
                nc.vector.memset(ones_row[:], 1.0)

            # --- x_static -> transposed [27, 128] + ones row 27 ---
            xst32 = stage.tile([B_L, STATIC], f32)
            nc.sync.dma_start(xst32[:], x_st[:])
            xst16 = stage.tile([B_L, 128], f16)
            nc.vector.memset(xst16[:], 0.0)
            nc.vector.tensor_copy(xst16[:, 0:STATIC], xst32[:])
            # ones in column 27 become the ones row after the transpose
            nc.vector.memset(xst16[:, STATIC : STATIC + 1], 1.0)
            nc.sync.dma_start_transpose(xsT[:], xst16[:])

            masks.make_identity(nc, ident[:])

            # --- i_gate = sigmoid(x_static @ W_sh + bias_s) ---
            ig_ps = psum_g.tile([128, U], f32, tag="ps_g")
            nc.tensor.matmul(
                ig_ps[:], xsT[0 : STATIC + 1, :], Wshb[:], start=True, stop=True
            )
            nc.scalar.activation(igate[:], ig_ps[:], AF.Sigmoid)

        # --- recurrent state ---
        st = ctx.enter_context(tc.tile_pool(name="state", bufs=2))
        tmp = ctx.enter_context(tc.tile_pool(name="tmp", bufs=3))

        c_prev = st.tile([128, U], f32, tag="c")
        nc.vector.memset(c_prev[:], 0.0)
        hT0 = st.tile([128, B_L], f16, tag="h0")
        nc.vector.memset(hT0[:], 0.0)
        hT1 = st.tile([128, B_L], f16, tag="h1")
        nc.vector.memset(hT1[:], 0.0)

        def x_matmuls(t, ps3):
            g4 = 32 * (t % 4)
            cc = t // 4
            xt = xT4[g4 : g4 + 32, cc * B_L : (cc + 1) * B_L]
            wx = Wih4[g4 : g4 + 32, :]
            for k, ps in enumerate(ps3):
                nc.tensor.matmul(
                    ps[:],
                    xt,
                    wx[:, k * U : (k + 1) * U],
                    start=True,
                    stop=False,
                    tile_position=(g4, 0),
                )
                if has_bias:
                    nc.tensor.matmul(
                        ps[:],
                        ones_row[:],
                        bias16[:, k * U : (k + 1) * U],
                        start=False,
                        stop=False,
                    )

        # Warmup burst: ~16 back-to-back matmuls give the PE HAM monitor a
        # sustained-busy window so it lifts the clock gate to 2.4 GHz before
        # the scan starts; steady-state gaps are too short to re-throttle.
        for w in range(16):
            wp = psum_t.tile([128, 2 * U], f32, tag="pt")
            nc.tensor.matmul(wp[:], ident[:], Whh0[:, 0 : 2 * U], start=True, stop=True)

        def new_ps3():
            ps_f = psum_f.tile([128, U], f32, tag="ps_f")
            ps_o = psum_o.tile([128, U], f32, tag="ps_o")
            ps_g = psum_g.tile([128, U], f32, tag="ps_g")
            return (ps_f, ps_o, ps_g)

        # software-pipelined by one step: the x contributions for step t+1 are
        # issued during step t (they depend only on constants), filling PE idle
        # gaps and leaving only the h matmuls on the step-boundary path.
        ps3_n = new_ps3()
        x_matmuls(0, ps3_n)

        for t in range(T):
            last = t == T - 1
            ps_f, ps_o, ps_g = ps3_n
            # f first (gates the c chain), then g (feeds tanh), o last
            nc.tensor.matmul(ps_f[:], hT0[:], Whh0[:, 0:U], start=False, stop=False)
            nc.tensor.matmul(ps_f[:], hT1[:], Whh1[:, 0:U], start=False, stop=True)
            nc.tensor.matmul(
                ps_g[:], hT0[:], Whh0[:, 2 * U : 3 * U], start=False, stop=False
            )
            nc.tensor.matmul(
                ps_g[:], hT1[:], Whh1[:, 2 * U : 3 * U], start=False, stop=True
            )
            nc.tensor.matmul(
                ps_o[:], hT0[:], Whh0[:, U : 2 * U], start=False, stop=False
            )
            nc.tensor.matmul(
                ps_o[:], hT1[:], Whh1[:, U : 2 * U], start=False, stop=True
            )
            if not last:
                ps3_n = new_ps3()
                x_matmuls(t + 1, ps3_n)

            # sigmoid(f) alone first: it gates the c chain; sigmoid(o) is only
            # needed ~1.5us later for the h products.
            sf = tmp.tile([128, U], f16, tag="sf")
            nc.scalar.activation(sf[:], ps_f[:], AF.Sigmoid)
            tg = tmp.tile([128, U], f16, tag="tg")
            nc.scalar.activation(tg[:], ps_g[:], AF.Tanh)
            so = tmp.tile([128, U], f16, tag="so")
            nc.scalar.activation(so[:], ps_o[:], AF.Sigmoid)

            m1 = tmp.tile([128, U], f32, tag="m1")
            nc.vector.tensor_mul(m1[:], sf[:], c_prev[:])
            m2 = tmp.tile([128, U], f16, tag="m2")
            nc.vector.tensor_mul(m2[:], igate[:], tg[:])
            c_new = st.tile([128, U], f32, tag="c")
            nc.vector.tensor_add(c_new[:], m1[:], m2[:])

            if last:
                tch = tmp.tile([128, U], f32, tag="tc32")
                nc.scalar.activation(tch[:], c_new[:], AF.Tanh)
                h_out = tmp.tile([128, U], f32, tag="hout")
                nc.vector.tensor_mul(h_out[:], so[:], tch[:])
                nc.sync.dma_start(out[:], h_out[:])
            else:
                # tail split into u-halves so transpose/copy/h-matmul of half 0
                # start while half 1 is still in ACT/DVE
                hTn = [None, None]
                for half in (0, 1):
                    lo, hi = 128 * half, 128 * (half + 1)
                    tch = tmp.tile([128, 128], f16, tag=f"tc{half}")
                    nc.scalar.activation(tch[:], c_new[:, lo:hi], AF.Tanh)
                    hh = tmp.tile([128, 128], f16, tag=f"hh{half}")
                    nc.vector.tensor_mul(hh[:], so[:, lo:hi], tch[:])
                    pp = psum_t.tile([128, 128], f16, tag="pt")
                    nc.tensor.transpose(pp[:], hh[:], ident[:])
                    ht_new = st.tile([128, B_L], f16, tag=f"h{half}")
                    nc.vector.tensor_copy(ht_new[:], pp[:])
                    hTn[half] = ht_new
                hT0, hT1 = hTn
            c_prev = c_new

    nc.compile()
    return nc


def get_program(has_bias: bool = False):
    if has_bias not in _cached:
        _cached[has_bias] = _build_program(has_bias)
    return _cached[has_bias]


def make_in_maps(inputs):
    x_dynamic = np.asarray(inputs["x_dynamic"], dtype=np.float32)
    x_static = np.asarray(inputs["x_static"], dtype=np.float32)
    w_ih = np.ascontiguousarray(np.asarray(inputs["weight_ih"], dtype=np.float32))
    w_hh = np.ascontiguousarray(np.asarray(inputs["weight_hh"], dtype=np.float32))
    w_sh = np.ascontiguousarray(np.asarray(inputs["weight_sh"], dtype=np.float32))
    bias = np.ascontiguousarray(
        np.asarray(inputs["bias"], dtype=np.float32).reshape(1, 3 * U)
    )
    bias_s = np.ascontiguousarray(
        np.asarray(inputs["bias_s"], dtype=np.float32).reshape(1, U)
    )
    in_maps = []
    for i in range(NCORES):
        sl = slice(i * B_L, (i + 1) * B_L)
        in_maps.append(
            {
                "x_dynamic": np.ascontiguousarray(
                    x_dynamic[sl].reshape(B_L, T * DYN)
                ),
                "x_static": np.ascontiguousarray(x_static[sl]),
                "weight_ih": w_ih,
                "weight_hh": w_hh,
                "weight_sh": w_sh,
                "bias": bias,
                "bias_s": bias_s,
            }
        )
    return in_maps


def kernel(**inputs) -> np.ndarray:
    from concourse.bass_utils import run_bass_kernel_spmd

    has_bias = bool(np.any(np.asarray(inputs["bias"])))
    nc = get_program(has_bias)
    in_maps = make_in_maps(inputs)
    res = run_bass_kernel_spmd(nc, in_maps, core_ids=list(range(NCORES)))
    return np.concatenate([r["out"] for r in res.results], axis=0).astype(np.float32)


# revision 26
# speedup vs baseline: 1.2435x; 1.0670x over previous
"""Trainium2 Bass kernel for EntityAwareLSTMLayer.

Problem (hardcoded):
  B=1024, T=365, DYN=32, STATIC=27, UNITS=256
  i_gate = sigmoid(x_static @ W_sh + bias_s)            [B, U]   (static, once)
  gx_t   = x_t @ W_ih + bias                            [B, 3U]
  gates  = gx_t + h @ W_hh                              [B, 3U]  (f|o|g)
  c      = sigmoid(f) * c + i_gate * tanh(g)
  h      = sigmoid(o) * tanh(c)
  return h_final                                        [B, U]

Sharding: data-parallel over batch, 8 cores x 128 rows. Batch rows live on
the 128 SBUF partitions; per step the gates are computed by PE matmuls
accumulating three K-chunks into PSUM: the x chunk (K=32) and two h chunks
(K=128 each, h stored transposed). fp16 matmul operands (1 cycle/row on PE
vs 4 for fp32), fp32 PSUM accumulation, fp32 cell state.

x_dynamic is transposed on-chip via DMA-xbar transposes of [128,128] fp16
chunks (4 timesteps per chunk); timestep t lands at partition group
32*(t%4), so W_ih is replicated at the 4 partition bases.
"""

import numpy as np

B_L = 128  # batch rows per core
T = 365
TP = 368  # T padded to a multiple of 4 for chunked transposes
DYN = 32
STATIC = 27
U = 256
NCORES = 8

_cached = {}


def _build_program(has_bias: bool):
    from contextlib import ExitStack

    import concourse.bacc as bacc
    import concourse.masks as masks
    import concourse.tile as tile
    from concourse import mybir

    f32 = mybir.dt.float32
    f16 = mybir.dt.float16
    AF = mybir.ActivationFunctionType

    nc = bacc.Bacc("TRN2", target_bir_lowering=False, debug=False)

    x_dyn = nc.dram_tensor("x_dynamic", [B_L, T * DYN], f32, kind="ExternalInput")
    x_st = nc.dram_tensor("x_static", [B_L, STATIC], f32, kind="ExternalInput")
    w_ih = nc.dram_tensor("weight_ih", [DYN, 3 * U], f32, kind="ExternalInput")
    w_hh = nc.dram_tensor("weight_hh", [U, 3 * U], f32, kind="ExternalInput")
    w_sh = nc.dram_tensor("weight_sh", [STATIC, U], f32, kind="ExternalInput")
    bias = nc.dram_tensor("bias", [1, 3 * U], f32, kind="ExternalInput")
    bias_s = nc.dram_tensor("bias_s", [1, U], f32, kind="ExternalInput")
    out = nc.dram_tensor("out", [B_L, U], f32, kind="ExternalOutput")

    with tile.TileContext(nc) as tc, ExitStack() as ctx:
        const = ctx.enter_context(tc.tile_pool(name="const", bufs=1))
        # x transposed in 4-timestep chunks: chunk c = timesteps 4c..4c+3,
        # partition 32*(t%4)+k holds feature k of timestep t, free col = batch.
        xT4 = const.tile([128, (TP // 4) * B_L], f16)
        Wih4 = const.tile([128, 3 * U], f16)  # W_ih replicated at 4 bases
        Whh0 = const.tile([128, 3 * U], f16)
        Whh1 = const.tile([128, 3 * U], f16)
        Wshb = const.tile([STATIC + 1, U], f16)  # rows 0-26 W_sh, row 27 bias_s
        xsT = const.tile([128, B_L], f16)
        ident = const.tile([128, 128], f16)
        igate = const.tile([128, U], f16)
        if has_bias:
            ones_row = const.tile([1, B_L], f16)
            bias16 = const.tile([1, 3 * U], f16)

        psum_f = ctx.enter_context(tc.tile_pool(name="pf", bufs=2, space="PSUM"))
        psum_o = ctx.enter_context(tc.tile_pool(name="po", bufs=2, space="PSUM"))
        psum_g = ctx.enter_context(tc.tile_pool(name="pg", bufs=2, space="PSUM"))
        psum_t = ctx.enter_context(tc.tile_pool(name="pt", bufs=2, space="PSUM"))

        with tc.tile_pool(name="stage", bufs=1) as stage:
            # --- x_dynamic: load fp32, convert fp16, transpose in chunks ---
            xs32 = stage.tile([B_L, T * DYN], f32)
            nc.sync.dma_start(xs32[:], x_dyn[:])
            x16 = stage.tile([B_L, TP * DYN], f16)
            nc.vector.memset(x16[:, T * DYN :], 0.0)
            nc.vector.tensor_copy(x16[:, 0 : T * DYN], xs32[:])
            for c in range(TP // 4):
                nc.sync.dma_start_transpose(
                    xT4[:, c * B_L : (c + 1) * B_L],
                    x16[:, c * 128 : (c + 1) * 128],
                )

            # --- weights ---
            wst = stage.tile([128, 3 * U], f32)
            nc.sync.dma_start(wst[:], w_hh[0:128, :])
            nc.vector.tensor_copy(Whh0[:], wst[:])
            nc.sync.dma_start(wst[:], w_hh[128:256, :])
            nc.vector.tensor_copy(Whh1[:], wst[:])
            wih32 = stage.tile([DYN, 3 * U], f32)
            nc.sync.dma_start(wih32[:], w_ih[:])
            for g in range(4):
                nc.vector.tensor_copy(Wih4[32 * g : 32 * g + 32, :], wih32[:])
            wsh32 = stage.tile([STATIC, U], f32)
            nc.sync.dma_start(wsh32[:], w_sh[:])
            nc.vector.tensor_copy(Wshb[0:STATIC, :], wsh32[:])
            bs32 = stage.tile([1, U], f32)
            nc.sync.dma_start(bs32[:], bias_s[:])
            bs16 = stage.tile([1, U], f16)
            nc.vector.tensor_copy(bs16[:], bs32[:])
            # partition 27 is not engine-addressable; DMA has no such limit
            nc.sync.dma_start(Wshb[STATIC : STATIC + 1, :], bs16[:])
            if has_bias:
                b32 = stage.tile([1, 3 * U], f32)
                nc.sync.dma_start(b32[:], bias[:])
                nc.vector.tensor_copy(bias16[:], b32[:])
                nc.vector.memset(ones_row[:], 1.0)

            # --- x_static -> transposed [27, 128] + ones row 27 ---
            xst32 = stage.tile([B_L, STATIC], f32)
            nc.sync.dma_start(xst32[:], x_st[:])
            xst16 = stage.tile([B_L, 128], f16)
            nc.vector.memset(xst16[:], 0.0)
            nc.vector.tensor_copy(xst16[:, 0:STATIC], xst32[:])
            # ones in column 27 become the ones row after the transpose
            nc.vector.memset(xst16[:, STATIC : STATIC + 1], 1.0)
            nc.sync.dma_start_transpose(xsT[:], xst16[:])

            masks.make_identity(nc, ident[:])

            # --- i_gate = sigmoid(x_static @ W_sh + bias_s) ---
            ig_ps = psum_g.tile([128, U], f32, tag="ps_g")
            nc.tensor.matmul(
                ig_ps[:], xsT[0 : STATIC + 1, :], Wshb[:], start=True, stop=True
            )
            nc.scalar.activation(igate[:], ig_ps[:], AF.Sigmoid)

        # --- recurrent state ---
        st = ctx.enter_context(tc.tile_pool(name="state", bufs=2))
        tmp = ctx.enter_context(tc.tile_pool(name="tmp", bufs=3))

        c_prev = st.tile([128, U], f32, tag="c")
        nc.vector.memset(c_prev[:], 0.0)
        hT0 = st.tile([128, B_L], f16, tag="h0")
        nc.vector.memset(hT0[:], 0.0)
        hT1 = st.tile([128, B_L], f16, tag="h1")
        nc.vector.memset(hT1[:], 0.0)

        def x_matmuls(t, ps3):
            g4 = 32 * (t % 4)
            cc = t // 4
            xt = xT4[g4 : g4 + 32, cc * B_L : (cc + 1) * B_L]
            wx = Wih4[g4 : g4 + 32, :]
            for k, ps in enumerate(ps3):
                nc.tensor.matmul(
                    ps[:],
                    xt,
                    wx[:, k * U : (k + 1) * U],
                    start=True,
                    stop=False,
                    tile_position=(g4, 0),
                )
                if has_bias:
                    nc.tensor.matmul(
                        ps[:],
                        ones_row[:],
                        bias16[:, k * U : (k + 1) * U],
                        start=False,
                        stop=False,
                    )

        # Warmup burst: ~16 back-to-back matmuls give the PE HAM monitor a
        # sustained-busy window so it lifts the clock gate to 2.4 GHz before
        # the scan starts; steady-state gaps are too short to re-throttle.
        for w in range(16):
            wp = psum_t.tile([128, 2 * U], f32, tag="pt")
            nc.tensor.matmul(wp[:], ident[:], Whh0[:, 0 : 2 * U], start=True, stop=True)

        def new_ps3():
            ps_f = psum_f.tile([128, U], f32, tag="ps_f")
            ps_o = psum_o.tile([128, U], f32, tag="ps_o")
            ps_g = psum_g.tile([128, U], f32, tag="ps_g")
            return (ps_f, ps_o, ps_g)

        # software-pipelined by one step: the x contributions for step t+1 are
        # issued during step t (they depend only on constants), filling PE idle
        # gaps and leaving only the h matmuls on the step-boundary path.
        ps3_n = new_ps3()
        x_matmuls(0, ps3_n)

        for t in range(T):
            last = t == T - 1
            ps_f, ps_o, ps_g = ps3_n
            # f first (gates the c chain), then g (feeds tanh), o last; the
            # explicit order-deps stop the scheduler from grouping all hT0
            # matmuls ahead of the hT1 ones (which would delay ps_f, and with
            # it sigmoid(f), by two extra matmuls)
            mms = []
            mms.append(
                nc.tensor.matmul(
                    ps_f[:], hT0[:], Whh0[:, 0:U], start=False, stop=False
                )
            )
            mms.append(
                nc.tensor.matmul(
                    ps_f[:], hT1[:], Whh1[:, 0:U], start=False, stop=True
                )
            )
            mms.append(
                nc.tensor.matmul(
                    ps_g[:], hT0[:], Whh0[:, 2 * U : 3 * U], start=False, stop=False
                )
            )
            mms.append(
                nc.tensor.matmul(
                    ps_g[:], hT1[:], Whh1[:, 2 * U : 3 * U], start=False, stop=True
                )
            )
            mms.append(
                nc.tensor.matmul(
                    ps_o[:], hT0[:], Whh0[:, U : 2 * U], start=False, stop=False
                )
            )
            mms.append(
                nc.tensor.matmul(
                    ps_o[:], hT1[:], Whh1[:, U : 2 * U], start=False, stop=True
                )
            )
            for a, b in zip(mms[1:], mms[:-1]):
                tile.add_dep_helper(
                    a.ins, b.ins, sync=False, reason="keep f,g,o MM order"
                )
            if not last:
                ps3_n = new_ps3()
                x_matmuls(t + 1, ps3_n)

            # sigmoid(f) alone first: it gates the c chain; sigmoid(o) is only
            # needed ~1.5us later for the h products.
            sf = tmp.tile([128, U], f16, tag="sf")
            nc.scalar.activation(sf[:], ps_f[:], AF.Sigmoid)
            tg = tmp.tile([128, U], f16, tag="tg")
            nc.scalar.activation(tg[:], ps_g[:], AF.Tanh)
            so = tmp.tile([128, U], f16, tag="so")
            nc.scalar.activation(so[:], ps_o[:], AF.Sigmoid)

            m1 = tmp.tile([128, U], f32, tag="m1")
            nc.vector.tensor_mul(m1[:], sf[:], c_prev[:])
            m2 = tmp.tile([128, U], f16, tag="m2")
            nc.vector.tensor_mul(m2[:], igate[:], tg[:])
            c_new = st.tile([128, U], f32, tag="c")
            nc.vector.tensor_add(c_new[:], m1[:], m2[:])

            if last:
                tch = tmp.tile([128, U], f32, tag="tc32")
                nc.scalar.activation(tch[:], c_new[:], AF.Tanh)
                h_out = tmp.tile([128, U], f32, tag="hout")
                nc.vector.tensor_mul(h_out[:], so[:], tch[:])
                nc.sync.dma_start(out[:], h_out[:])
            else:
                # tail split into u-halves so transpose/copy/h-matmul of half 0
                # start while half 1 is still in ACT/DVE
                hTn = [None, None]
                for half in (0, 1):
                    lo, hi = 128 * half, 128 * (half + 1)
                    tch = tmp.tile([128, 128], f16, tag=f"tc{half}")
                    nc.scalar.activation(tch[:], c_new[:, lo:hi], AF.Tanh)
                    hh = tmp.tile([128, 128], f16, tag=f"hh{half}")
                    nc.vector.tensor_mul(hh[:], so[:, lo:hi], tch[:])
                    pp = psum_t.tile([128, 128], f16, tag="pt")
                    nc.tensor.transpose(pp[:], hh[:], ident[:])
                    ht_new = st.tile([128, B_L], f16, tag=f"h{half}")
                    nc.vector.tensor_copy(ht_new[:], pp[:])
                    hTn[half] = ht_new
                hT0, hT1 = hTn
            c_prev = c_new

    nc.compile()
    return nc


def get_program(has_bias: bool = False):
    if has_bias not in _cached:
        _cached[has_bias] = _build_program(has_bias)
    return _cached[has_bias]


def make_in_maps(inputs):
    x_dynamic = np.asarray(inputs["x_dynamic"], dtype=np.float32)
    x_static = np.asarray(inputs["x_static"], dtype=np.float32)
    w_ih = np.ascontiguousarray(np.asarray(inputs["weight_ih"], dtype=np.float32))
    w_hh = np.ascontiguousarray(np.asarray(inputs["weight_hh"], dtype=np.float32))
    w_sh = np.ascontiguousarray(np.asarray(inputs["weight_sh"], dtype=np.float32))
    bias = np.ascontiguousarray(
        np.asarray(inputs["bias"], dtype=np.float32).reshape(1, 3 * U)
    )
    bias_s = np.ascontiguousarray(
        np.asarray(inputs["bias_s"], dtype=np.float32).reshape(1, U)
    )
    in_maps = []
    for i in range(NCORES):
        sl = slice(i * B_L, (i + 1) * B_L)
        in_maps.append(
            {
                "x_dynamic": np.ascontiguousarray(
                    x_dynamic[sl].reshape(B_L, T * DYN)
                ),
                "x_static": np.ascontiguousarray(x_static[sl]),
                "weight_ih": w_ih,
                "weight_hh": w_hh,
                "weight_sh": w_sh,
                "bias": bias,
                "bias_s": bias_s,
            }
        )
    return in_maps


def kernel(**inputs) -> np.ndarray:
    from concourse.bass_utils import run_bass_kernel_spmd

    has_bias = bool(np.any(np.asarray(inputs["bias"])))
    nc = get_program(has_bias)
    in_maps = make_in_maps(inputs)
    res = run_bass_kernel_spmd(nc, in_maps, core_ids=list(range(NCORES)))
    return np.concatenate([r["out"] for r in res.results], axis=0).astype(np.float32)


# revision 27
# speedup vs baseline: 1.2812x; 1.0304x over previous
"""Trainium2 Bass kernel for EntityAwareLSTMLayer.

Problem (hardcoded):
  B=1024, T=365, DYN=32, STATIC=27, UNITS=256
  i_gate = sigmoid(x_static @ W_sh + bias_s)            [B, U]   (static, once)
  gx_t   = x_t @ W_ih + bias                            [B, 3U]
  gates  = gx_t + h @ W_hh                              [B, 3U]  (f|o|g)
  c      = sigmoid(f) * c + i_gate * tanh(g)
  h      = sigmoid(o) * tanh(c)
  return h_final                                        [B, U]

Sharding: data-parallel over batch, 8 cores x 128 rows. Batch rows live on
the 128 SBUF partitions; per step the gates are computed by PE matmuls
accumulating three K-chunks into PSUM: the x chunk (K=32) and two h chunks
(K=128 each, h stored transposed). fp16 matmul operands (1 cycle/row on PE
vs 4 for fp32), fp32 PSUM accumulation, fp32 cell state.

x_dynamic is transposed on-chip via DMA-xbar transposes of [128,128] fp16
chunks (4 timesteps per chunk); timestep t lands at partition group
32*(t%4), so W_ih is replicated at the 4 partition bases.
"""

import numpy as np

B_L = 128  # batch rows per core
T = 365
TP = 368  # T padded to a multiple of 4 for chunked transposes
DYN = 32
STATIC = 27
U = 256
NCORES = 8

_cached = {}


def _build_program(has_bias: bool):
    from contextlib import ExitStack

    import concourse.bacc as bacc
    import concourse.masks as masks
    import concourse.tile as tile
    from concourse import mybir

    f32 = mybir.dt.float32
    f16 = mybir.dt.float16
    AF = mybir.ActivationFunctionType

    nc = bacc.Bacc("TRN2", target_bir_lowering=False, debug=False)

    x_dyn = nc.dram_tensor("x_dynamic", [B_L, T * DYN], f32, kind="ExternalInput")
    x_st = nc.dram_tensor("x_static", [B_L, STATIC], f32, kind="ExternalInput")
    w_ih = nc.dram_tensor("weight_ih", [DYN, 3 * U], f32, kind="ExternalInput")
    w_hh = nc.dram_tensor("weight_hh", [U, 3 * U], f32, kind="ExternalInput")
    w_sh = nc.dram_tensor("weight_sh", [STATIC, U], f32, kind="ExternalInput")
    bias = nc.dram_tensor("bias", [1, 3 * U], f32, kind="ExternalInput")
    bias_s = nc.dram_tensor("bias_s", [1, U], f32, kind="ExternalInput")
    out = nc.dram_tensor("out", [B_L, U], f32, kind="ExternalOutput")

    with tile.TileContext(nc) as tc, ExitStack() as ctx:
        const = ctx.enter_context(tc.tile_pool(name="const", bufs=1))
        # x transposed in 4-timestep chunks: chunk c = timesteps 4c..4c+3,
        # partition 32*(t%4)+k holds feature k of timestep t, free col = batch.
        xT4 = const.tile([128, (TP // 4) * B_L], f16)
        Wih4 = const.tile([128, 3 * U], f16)  # W_ih replicated at 4 bases
        Whh0 = const.tile([128, 3 * U], f16)
        Whh1 = const.tile([128, 3 * U], f16)
        Wshb = const.tile([STATIC + 1, U], f16)  # rows 0-26 W_sh, row 27 bias_s
        xsT = const.tile([128, B_L], f16)
        ident = const.tile([128, 128], f16)
        igate = const.tile([128, U], f16)
        if has_bias:
            ones_row = const.tile([1, B_L], f16)
            bias16 = const.tile([1, 3 * U], f16)

        psum_f = ctx.enter_context(tc.tile_pool(name="pf", bufs=2, space="PSUM"))
        psum_o = ctx.enter_context(tc.tile_pool(name="po", bufs=2, space="PSUM"))
        psum_g = ctx.enter_context(tc.tile_pool(name="pg", bufs=2, space="PSUM"))
        psum_t = ctx.enter_context(tc.tile_pool(name="pt", bufs=2, space="PSUM"))

        with tc.tile_pool(name="stage", bufs=1) as stage:
            # --- x_dynamic: load fp32, convert fp16, transpose in chunks ---
            xs32 = stage.tile([B_L, T * DYN], f32)
            nc.sync.dma_start(xs32[:], x_dyn[:])
            x16 = stage.tile([B_L, TP * DYN], f16)
            nc.vector.memset(x16[:, T * DYN :], 0.0)
            nc.vector.tensor_copy(x16[:, 0 : T * DYN], xs32[:])
            for c in range(TP // 4):
                nc.sync.dma_start_transpose(
                    xT4[:, c * B_L : (c + 1) * B_L],
                    x16[:, c * 128 : (c + 1) * 128],
                )

            # --- weights ---
            wst = stage.tile([128, 3 * U], f32)
            nc.sync.dma_start(wst[:], w_hh[0:128, :])
            nc.vector.tensor_copy(Whh0[:], wst[:])
            nc.sync.dma_start(wst[:], w_hh[128:256, :])
            nc.vector.tensor_copy(Whh1[:], wst[:])
            wih32 = stage.tile([DYN, 3 * U], f32)
            nc.sync.dma_start(wih32[:], w_ih[:])
            for g in range(4):
                nc.vector.tensor_copy(Wih4[32 * g : 32 * g + 32, :], wih32[:])
            wsh32 = stage.tile([STATIC, U], f32)
            nc.sync.dma_start(wsh32[:], w_sh[:])
            nc.vector.tensor_copy(Wshb[0:STATIC, :], wsh32[:])
            bs32 = stage.tile([1, U], f32)
            nc.sync.dma_start(bs32[:], bias_s[:])
            bs16 = stage.tile([1, U], f16)
            nc.vector.tensor_copy(bs16[:], bs32[:])
            # partition 27 is not engine-addressable; DMA has no such limit
            nc.sync.dma_start(Wshb[STATIC : STATIC + 1, :], bs16[:])
            if has_bias:
                b32 = stage.tile([1, 3 * U], f32)
                nc.sync.dma_start(b32[:], bias[:])
                nc.vector.tensor_copy(bias16[:], b32[:])
                nc.vector.memset(ones_row[:], 1.0)

            # --- x_static -> transposed [27, 128] + ones row 27 ---
            xst32 = stage.tile([B_L, STATIC], f32)
            nc.sync.dma_start(xst32[:], x_st[:])
            xst16 = stage.tile([B_L, 128], f16)
            nc.vector.memset(xst16[:], 0.0)
            nc.vector.tensor_copy(xst16[:, 0:STATIC], xst32[:])
            # ones in column 27 become the ones row after the transpose
            nc.vector.memset(xst16[:, STATIC : STATIC + 1], 1.0)
            nc.sync.dma_start_transpose(xsT[:], xst16[:])

            masks.make_identity(nc, ident[:])

            # --- i_gate = sigmoid(x_static @ W_sh + bias_s) ---
            ig_ps = psum_g.tile([128, U], f32, tag="ps_g")
            nc.tensor.matmul(
                ig_ps[:], xsT[0 : STATIC + 1, :], Wshb[:], start=True, stop=True
            )
            nc.scalar.activation(igate[:], ig_ps[:], AF.Sigmoid)

        # --- recurrent state ---
        st = ctx.enter_context(tc.tile_pool(name="state", bufs=2))
        tmp = ctx.enter_context(tc.tile_pool(name="tmp", bufs=3))

        c_prev = st.tile([128, U], f16, tag="c")
        nc.vector.memset(c_prev[:], 0.0)
        hT0 = st.tile([128, B_L], f16, tag="h0")
        nc.vector.memset(hT0[:], 0.0)
        hT1 = st.tile([128, B_L], f16, tag="h1")
        nc.vector.memset(hT1[:], 0.0)

        def x_matmuls(t, ps3):
            g4 = 32 * (t % 4)
            cc = t // 4
            xt = xT4[g4 : g4 + 32, cc * B_L : (cc + 1) * B_L]
            wx = Wih4[g4 : g4 + 32, :]
            for k, ps in enumerate(ps3):
                nc.tensor.matmul(
                    ps[:],
                    xt,
                    wx[:, k * U : (k + 1) * U],
                    start=True,
                    stop=False,
                    tile_position=(g4, 0),
                )
                if has_bias:
                    nc.tensor.matmul(
                        ps[:],
                        ones_row[:],
                        bias16[:, k * U : (k + 1) * U],
                        start=False,
                        stop=False,
                    )

        # Warmup burst: ~16 back-to-back matmuls give the PE HAM monitor a
        # sustained-busy window so it lifts the clock gate to 2.4 GHz before
        # the scan starts; steady-state gaps are too short to re-throttle.
        for w in range(16):
            wp = psum_t.tile([128, 2 * U], f32, tag="pt")
            nc.tensor.matmul(wp[:], ident[:], Whh0[:, 0 : 2 * U], start=True, stop=True)

        def new_ps3():
            ps_f = psum_f.tile([128, U], f32, tag="ps_f")
            ps_o = psum_o.tile([128, U], f32, tag="ps_o")
            ps_g = psum_g.tile([128, U], f32, tag="ps_g")
            return (ps_f, ps_o, ps_g)

        # software-pipelined by one step: the x contributions for step t+1 are
        # issued during step t (they depend only on constants), filling PE idle
        # gaps and leaving only the h matmuls on the step-boundary path.
        ps3_n = new_ps3()
        x_matmuls(0, ps3_n)

        for t in range(T):
            last = t == T - 1
            ps_f, ps_o, ps_g = ps3_n
            # f first (gates the c chain), then g (feeds tanh), o last; the
            # explicit order-deps stop the scheduler from grouping all hT0
            # matmuls ahead of the hT1 ones (which would delay ps_f, and with
            # it sigmoid(f), by two extra matmuls)
            mms = []
            mms.append(
                nc.tensor.matmul(
                    ps_f[:], hT0[:], Whh0[:, 0:U], start=False, stop=False
                )
            )
            mms.append(
                nc.tensor.matmul(
                    ps_f[:], hT1[:], Whh1[:, 0:U], start=False, stop=True
                )
            )
            mms.append(
                nc.tensor.matmul(
                    ps_g[:], hT0[:], Whh0[:, 2 * U : 3 * U], start=False, stop=False
                )
            )
            mms.append(
                nc.tensor.matmul(
                    ps_g[:], hT1[:], Whh1[:, 2 * U : 3 * U], start=False, stop=True
                )
            )
            mms.append(
                nc.tensor.matmul(
                    ps_o[:], hT0[:], Whh0[:, U : 2 * U], start=False, stop=False
                )
            )
            mms.append(
                nc.tensor.matmul(
                    ps_o[:], hT1[:], Whh1[:, U : 2 * U], start=False, stop=True
                )
            )
            for a, b in zip(mms[1:], mms[:-1]):
                tile.add_dep_helper(
                    a.ins, b.ins, sync=False, reason="keep f,g,o MM order"
                )
            if not last:
                ps3_n = new_ps3()
                x_matmuls(t + 1, ps3_n)

            # sigmoid(f) alone first: it gates the c chain; sigmoid(o) is only
            # needed ~1.5us later for the h products.
            sf = tmp.tile([128, U], f16, tag="sf")
            nc.scalar.activation(sf[:], ps_f[:], AF.Sigmoid)
            tg = tmp.tile([128, U], f16, tag="tg")
            nc.scalar.activation(tg[:], ps_g[:], AF.Tanh)
            so = tmp.tile([128, U], f16, tag="so")
            nc.scalar.activation(so[:], ps_o[:], AF.Sigmoid)

            m1 = tmp.tile([128, U], f16, tag="m1")
            nc.vector.tensor_mul(m1[:], sf[:], c_prev[:])
            m2 = tmp.tile([128, U], f16, tag="m2")
            nc.vector.tensor_mul(m2[:], igate[:], tg[:])
            c_new = st.tile([128, U], f16, tag="c")
            nc.vector.tensor_add(c_new[:], m1[:], m2[:])

            if last:
                tch = tmp.tile([128, U], f32, tag="tc32")
                nc.scalar.activation(tch[:], c_new[:], AF.Tanh)
                h_out = tmp.tile([128, U], f32, tag="hout")
                nc.vector.tensor_mul(h_out[:], so[:], tch[:])
                nc.sync.dma_start(out[:], h_out[:])
            else:
                # tail split into u-halves so transpose/copy/h-matmul of half 0
                # start while half 1 is still in ACT/DVE
                hTn = [None, None]
                for half in (0, 1):
                    lo, hi = 128 * half, 128 * (half + 1)
                    tch = tmp.tile([128, 128], f16, tag=f"tc{half}")
                    nc.scalar.activation(tch[:], c_new[:, lo:hi], AF.Tanh)
                    hh = tmp.tile([128, 128], f16, tag=f"hh{half}")
                    nc.vector.tensor_mul(hh[:], so[:, lo:hi], tch[:])
                    pp = psum_t.tile([128, 128], f16, tag="pt")
                    nc.tensor.transpose(pp[:], hh[:], ident[:])
                    ht_new = st.tile([128, B_L], f16, tag=f"h{half}")
                    nc.vector.tensor_copy(ht_new[:], pp[:])
                    hTn[half] = ht_new
                hT0, hT1 = hTn
            c_prev = c_new

    nc.compile()
    return nc


def get_program(has_bias: bool = False):
    if has_bias not in _cached:
        _cached[has_bias] = _build_program(has_bias)
    return _cached[has_bias]


def make_in_maps(inputs):
    x_dynamic = np.asarray(inputs["x_dynamic"], dtype=np.float32)
    x_static = np.asarray(inputs["x_static"], dtype=np.float32)
    w_ih = np.ascontiguousarray(np.asarray(inputs["weight_ih"], dtype=np.float32))
    w_hh = np.ascontiguousarray(np.asarray(inputs["weight_hh"], dtype=np.float32))
    w_sh = np.ascontiguousarray(np.asarray(inputs["weight_sh"], dtype=np.float32))
    bias = np.ascontiguousarray(
        np.asarray(inputs["bias"], dtype=np.float32).reshape(1, 3 * U)
    )
    bias_s = np.ascontiguousarray(
        np.asarray(inputs["bias_s"], dtype=np.float32).reshape(1, U)
    )
    in_maps = []
    for i in range(NCORES):
        sl = slice(i * B_L, (i + 1) * B_L)
        in_maps.append(
            {
                "x_dynamic": np.ascontiguousarray(
                    x_dynamic[sl].reshape(B_L, T * DYN)
                ),
                "x_static": np.ascontiguousarray(x_static[sl]),
                "weight_ih": w_ih,
                "weight_hh": w_hh,
                "weight_sh": w_sh,
                "bias": bias,
                "bias_s": bias_s,
            }
        )
    return in_maps


def kernel(**inputs) -> np.ndarray:
    from concourse.bass_utils import run_bass_kernel_spmd

    has_bias = bool(np.any(np.asarray(inputs["bias"])))
    nc = get_program(has_bias)
    in_maps = make_in_maps(inputs)
    res = run_bass_kernel_spmd(nc, in_maps, core_ids=list(range(NCORES)))
    return np.concatenate([r["out"] for r in res.results], axis=0).astype(np.float32)


# revision 28
# speedup vs baseline: 1.2849x; 1.0028x over previous
"""Trainium2 Bass kernel for EntityAwareLSTMLayer.

Problem (hardcoded):
  B=1024, T=365, DYN=32, STATIC=27, UNITS=256
  i_gate = sigmoid(x_static @ W_sh + bias_s)            [B, U]   (static, once)
  gx_t   = x_t @ W_ih + bias                            [B, 3U]
  gates  = gx_t + h @ W_hh                              [B, 3U]  (f|o|g)
  c      = sigmoid(f) * c + i_gate * tanh(g)
  h      = sigmoid(o) * tanh(c)
  return h_final                                        [B, U]

Sharding: data-parallel over batch, 8 cores x 128 rows. Batch rows live on
the 128 SBUF partitions; per step the gates are computed by PE matmuls
accumulating three K-chunks into PSUM: the x chunk (K=32) and two h chunks
(K=128 each, h stored transposed). fp16 matmul operands (1 cycle/row on PE
vs 4 for fp32), fp32 PSUM accumulation, fp32 cell state.

x_dynamic is transposed on-chip via DMA-xbar transposes of [128,128] fp16
chunks (4 timesteps per chunk); timestep t lands at partition group
32*(t%4), so W_ih is replicated at the 4 partition bases.
"""

import numpy as np

B_L = 128  # batch rows per core
T = 365
TP = 368  # T padded to a multiple of 4 for chunked transposes
DYN = 32
STATIC = 27
U = 256
NCORES = 8

_cached = {}


def _build_program(has_bias: bool):
    from contextlib import ExitStack

    import concourse.bacc as bacc
    import concourse.masks as masks
    import concourse.tile as tile
    from concourse import mybir

    f32 = mybir.dt.float32
    f16 = mybir.dt.float16
    AF = mybir.ActivationFunctionType

    nc = bacc.Bacc("TRN2", target_bir_lowering=False, debug=False)

    x_dyn = nc.dram_tensor("x_dynamic", [B_L, T * DYN], f32, kind="ExternalInput")
    x_st = nc.dram_tensor("x_static", [B_L, STATIC], f32, kind="ExternalInput")
    w_ih = nc.dram_tensor("weight_ih", [DYN, 3 * U], f32, kind="ExternalInput")
    w_hh = nc.dram_tensor("weight_hh", [U, 3 * U], f32, kind="ExternalInput")
    w_sh = nc.dram_tensor("weight_sh", [STATIC, U], f32, kind="ExternalInput")
    bias = nc.dram_tensor("bias", [1, 3 * U], f32, kind="ExternalInput")
    bias_s = nc.dram_tensor("bias_s", [1, U], f32, kind="ExternalInput")
    out = nc.dram_tensor("out", [B_L, U], f32, kind="ExternalOutput")

    with tile.TileContext(nc) as tc, ExitStack() as ctx:
        const = ctx.enter_context(tc.tile_pool(name="const", bufs=1))
        # x transposed in 4-timestep chunks: chunk c = timesteps 4c..4c+3,
        # partition 32*(t%4)+k holds feature k of timestep t, free col = batch.
        xT4 = const.tile([128, (TP // 4) * B_L], f16)
        Wih4 = const.tile([128, 3 * U], f16)  # W_ih replicated at 4 bases
        Whh0 = const.tile([128, 3 * U], f16)
        Whh1 = const.tile([128, 3 * U], f16)
        Wshb = const.tile([STATIC + 1, U], f16)  # rows 0-26 W_sh, row 27 bias_s
        xsT = const.tile([128, B_L], f16)
        ident = const.tile([128, 128], f16)
        igate = const.tile([128, U], f16)
        if has_bias:
            ones_row = const.tile([1, B_L], f16)
            bias16 = const.tile([1, 3 * U], f16)

        psum_f = ctx.enter_context(tc.tile_pool(name="pf", bufs=2, space="PSUM"))
        psum_o = ctx.enter_context(tc.tile_pool(name="po", bufs=2, space="PSUM"))
        psum_g = ctx.enter_context(tc.tile_pool(name="pg", bufs=2, space="PSUM"))
        psum_t = ctx.enter_context(tc.tile_pool(name="pt", bufs=2, space="PSUM"))

        with tc.tile_pool(name="stage", bufs=1) as stage:
            # --- weights (small, fast — do these first so the scan can start
            # as soon as the leading x chunks are transposed) ---
            wst = stage.tile([128, 3 * U], f32)
            nc.sync.dma_start(wst[:], w_hh[0:128, :])
            nc.vector.tensor_copy(Whh0[:], wst[:])
            nc.sync.dma_start(wst[:], w_hh[128:256, :])
            nc.vector.tensor_copy(Whh1[:], wst[:])
            wih32 = stage.tile([DYN, 3 * U], f32)
            nc.sync.dma_start(wih32[:], w_ih[:])
            for g in range(4):
                nc.vector.tensor_copy(Wih4[32 * g : 32 * g + 32, :], wih32[:])
            wsh32 = stage.tile([STATIC, U], f32)
            nc.sync.dma_start(wsh32[:], w_sh[:])
            nc.vector.tensor_copy(Wshb[0:STATIC, :], wsh32[:])
            bs32 = stage.tile([1, U], f32)
            nc.sync.dma_start(bs32[:], bias_s[:])
            bs16 = stage.tile([1, U], f16)
            nc.vector.tensor_copy(bs16[:], bs32[:])
            # partition 27 is not engine-addressable; DMA has no such limit
            nc.sync.dma_start(Wshb[STATIC : STATIC + 1, :], bs16[:])
            if has_bias:
                b32 = stage.tile([1, 3 * U], f32)
                nc.sync.dma_start(b32[:], bias[:])
                nc.vector.tensor_copy(bias16[:], b32[:])
                nc.vector.memset(ones_row[:], 1.0)

            # --- x_static -> transposed [27, 128] + ones row 27 ---
            xst32 = stage.tile([B_L, STATIC], f32)
            nc.sync.dma_start(xst32[:], x_st[:])
            xst16 = stage.tile([B_L, 128], f16)
            nc.vector.memset(xst16[:], 0.0)
            nc.vector.tensor_copy(xst16[:, 0:STATIC], xst32[:])
            # ones in column 27 become the ones row after the transpose
            nc.vector.memset(xst16[:, STATIC : STATIC + 1], 1.0)
            nc.sync.dma_start_transpose(xsT[:], xst16[:])

            masks.make_identity(nc, ident[:])

            # --- i_gate = sigmoid(x_static @ W_sh + bias_s) ---
            ig_ps = psum_g.tile([128, U], f32, tag="ps_g")
            nc.tensor.matmul(
                ig_ps[:], xsT[0 : STATIC + 1, :], Wshb[:], start=True, stop=True
            )
            nc.scalar.activation(igate[:], ig_ps[:], AF.Sigmoid)

            # --- x_dynamic: load fp32, convert fp16, transpose in chunks;
            # staged in two pieces so the first timesteps are available within
            # a few microseconds while the bulk streams in behind the scan ---
            x16 = stage.tile([B_L, TP * DYN], f16)
            nc.vector.memset(x16[:, T * DYN :], 0.0)
            NCH = 8  # transpose chunks (4 timesteps each) in the first piece
            split = NCH * 128
            xs32a = stage.tile([B_L, split], f32)
            nc.sync.dma_start(xs32a[:], x_dyn[:, 0:split])
            nc.vector.tensor_copy(x16[:, 0:split], xs32a[:])
            for c in range(NCH):
                nc.sync.dma_start_transpose(
                    xT4[:, c * B_L : (c + 1) * B_L],
                    x16[:, c * 128 : (c + 1) * 128],
                )
            xs32b = stage.tile([B_L, T * DYN - split], f32)
            nc.sync.dma_start(xs32b[:], x_dyn[:, split:])
            nc.vector.tensor_copy(x16[:, split : T * DYN], xs32b[:])
            for c in range(NCH, TP // 4):
                nc.sync.dma_start_transpose(
                    xT4[:, c * B_L : (c + 1) * B_L],
                    x16[:, c * 128 : (c + 1) * 128],
                )

        # --- recurrent state ---
        st = ctx.enter_context(tc.tile_pool(name="state", bufs=2))
        tmp = ctx.enter_context(tc.tile_pool(name="tmp", bufs=3))

        c_prev = st.tile([128, U], f16, tag="c")
        nc.vector.memset(c_prev[:], 0.0)
        hT0 = st.tile([128, B_L], f16, tag="h0")
        nc.vector.memset(hT0[:], 0.0)
        hT1 = st.tile([128, B_L], f16, tag="h1")
        nc.vector.memset(hT1[:], 0.0)

        def x_matmuls(t, ps3):
            g4 = 32 * (t % 4)
            cc = t // 4
            xt = xT4[g4 : g4 + 32, cc * B_L : (cc + 1) * B_L]
            wx = Wih4[g4 : g4 + 32, :]
            for k, ps in enumerate(ps3):
                nc.tensor.matmul(
                    ps[:],
                    xt,
                    wx[:, k * U : (k + 1) * U],
                    start=True,
                    stop=False,
                    tile_position=(g4, 0),
                )
                if has_bias:
                    nc.tensor.matmul(
                        ps[:],
                        ones_row[:],
                        bias16[:, k * U : (k + 1) * U],
                        start=False,
                        stop=False,
                    )

        # Warmup burst: ~16 back-to-back matmuls give the PE HAM monitor a
        # sustained-busy window so it lifts the clock gate to 2.4 GHz before
        # the scan starts; steady-state gaps are too short to re-throttle.
        for w in range(16):
            wp = psum_t.tile([128, 2 * U], f32, tag="pt")
            nc.tensor.matmul(wp[:], ident[:], Whh0[:, 0 : 2 * U], start=True, stop=True)

        def new_ps3():
            ps_f = psum_f.tile([128, U], f32, tag="ps_f")
            ps_o = psum_o.tile([128, U], f32, tag="ps_o")
            ps_g = psum_g.tile([128, U], f32, tag="ps_g")
            return (ps_f, ps_o, ps_g)

        # software-pipelined by one step: the x contributions for step t+1 are
        # issued during step t (they depend only on constants), filling PE idle
        # gaps and leaving only the h matmuls on the step-boundary path.
        ps3_n = new_ps3()
        x_matmuls(0, ps3_n)

        for t in range(T):
            last = t == T - 1
            ps_f, ps_o, ps_g = ps3_n
            # f first (gates the c chain), then g (feeds tanh), o last; the
            # explicit order-deps stop the scheduler from grouping all hT0
            # matmuls ahead of the hT1 ones (which would delay ps_f, and with
            # it sigmoid(f), by two extra matmuls)
            mms = []
            mms.append(
                nc.tensor.matmul(
                    ps_f[:], hT0[:], Whh0[:, 0:U], start=False, stop=False
                )
            )
            mms.append(
                nc.tensor.matmul(
                    ps_f[:], hT1[:], Whh1[:, 0:U], start=False, stop=True
                )
            )
            mms.append(
                nc.tensor.matmul(
                    ps_g[:], hT0[:], Whh0[:, 2 * U : 3 * U], start=False, stop=False
                )
            )
            mms.append(
                nc.tensor.matmul(
                    ps_g[:], hT1[:], Whh1[:, 2 * U : 3 * U], start=False, stop=True
                )
            )
            mms.append(
                nc.tensor.matmul(
                    ps_o[:], hT0[:], Whh0[:, U : 2 * U], start=False, stop=False
                )
            )
            mms.append(
                nc.tensor.matmul(
                    ps_o[:], hT1[:], Whh1[:, U : 2 * U], start=False, stop=True
                )
            )
            for a, b in zip(mms[1:], mms[:-1]):
                tile.add_dep_helper(
                    a.ins, b.ins, sync=False, reason="keep f,g,o MM order"
                )
            if not last:
                ps3_n = new_ps3()
                x_matmuls(t + 1, ps3_n)

            # sigmoid(f) alone first: it gates the c chain; sigmoid(o) is only
            # needed ~1.5us later for the h products.
            sf = tmp.tile([128, U], f16, tag="sf")
            nc.scalar.activation(sf[:], ps_f[:], AF.Sigmoid)
            tg = tmp.tile([128, U], f16, tag="tg")
            nc.scalar.activation(tg[:], ps_g[:], AF.Tanh)
            so = tmp.tile([128, U], f16, tag="so")
            nc.scalar.activation(so[:], ps_o[:], AF.Sigmoid)

            m1 = tmp.tile([128, U], f16, tag="m1")
            nc.vector.tensor_mul(m1[:], sf[:], c_prev[:])
            m2 = tmp.tile([128, U], f16, tag="m2")
            nc.vector.tensor_mul(m2[:], igate[:], tg[:])
            c_new = st.tile([128, U], f16, tag="c")
            nc.vector.tensor_add(c_new[:], m1[:], m2[:])

            if last:
                tch = tmp.tile([128, U], f32, tag="tc32")
                nc.scalar.activation(tch[:], c_new[:], AF.Tanh)
                h_out = tmp.tile([128, U], f32, tag="hout")
                nc.vector.tensor_mul(h_out[:], so[:], tch[:])
                nc.sync.dma_start(out[:], h_out[:])
            else:
                # tail split into u-halves so transpose/copy/h-matmul of half 0
                # start while half 1 is still in ACT/DVE
                hTn = [None, None]
                for half in (0, 1):
                    lo, hi = 128 * half, 128 * (half + 1)
                    tch = tmp.tile([128, 128], f16, tag=f"tc{half}")
                    nc.scalar.activation(tch[:], c_new[:, lo:hi], AF.Tanh)
                    hh = tmp.tile([128, 128], f16, tag=f"hh{half}")
                    nc.vector.tensor_mul(hh[:], so[:, lo:hi], tch[:])
                    pp = psum_t.tile([128, 128], f16, tag="pt")
                    nc.tensor.transpose(pp[:], hh[:], ident[:])
                    ht_new = st.tile([128, B_L], f16, tag=f"h{half}")
                    nc.vector.tensor_copy(ht_new[:], pp[:])
                    hTn[half] = ht_new
                hT0, hT1 = hTn
            c_prev = c_new

    nc.compile()
    return nc


def get_program(has_bias: bool = False):
    if has_bias not in _cached:
        _cached[has_bias] = _build_program(has_bias)
    return _cached[has_bias]


def make_in_maps(inputs):
    x_dynamic = np.asarray(inputs["x_dynamic"], dtype=np.float32)
    x_static = np.asarray(inputs["x_static"], dtype=np.float32)
    w_ih = np.ascontiguousarray(np.asarray(inputs["weight_ih"], dtype=np.float32))
    w_hh = np.ascontiguousarray(np.asarray(inputs["weight_hh"], dtype=np.float32))
    w_sh = np.ascontiguousarray(np.asarray(inputs["weight_sh"], dtype=np.float32))
    bias = np.ascontiguousarray(
        np.asarray(inputs["bias"], dtype=np.float32).reshape(1, 3 * U)
    )
    bias_s = np.ascontiguousarray(
        np.asarray(inputs["bias_s"], dtype=np.float32).reshape(1, U)
    )
    in_maps = []
    for i in range(NCORES):
        sl = slice(i * B_L, (i + 1) * B_L)
        in_maps.append(
            {
                "x_dynamic": np.ascontiguousarray(
                    x_dynamic[sl].reshape(B_L, T * DYN)
                ),
                "x_static": np.ascontiguousarray(x_static[sl]),
                "weight_ih": w_ih,
                "weight_hh": w_hh,
                "weight_sh": w_sh,
                "bias": bias,
                "bias_s": bias_s,
            }
        )
    return in_maps


def kernel(**inputs) -> np.ndarray:
    from concourse.bass_utils import run_bass_kernel_spmd

    has_bias = bool(np.any(np.asarray(inputs["bias"])))
    nc = get_program(has_bias)
    in_maps = make_in_maps(inputs)
    res = run_bass_kernel_spmd(nc, in_maps, core_ids=list(range(NCORES)))
    return np.concatenate([r["out"] for r in res.results], axis=0).astype(np.float32)
